# revision 4
# baseline (speedup 1.0000x reference)
"""Trainium2 Bass kernel for nn_DiffSCM: diffusion MLP sampler.

Data-parallel over 8 NeuronCores (batch sharding, 32768 rows/core).
Hybrid layout: prologue/epilogue in natural layout ([rows, D], rows on
partitions) so per-row vectors (sqrt(1-t), t_rand, ...) are per-partition
scalars fusable into single scalar_tensor_tensor DVE ops; the matmul chain
runs in transposed layout ([D, rows]) fed by PE transposes. Matmuls and
transposes use float32r (fast fp32, ~1e-4 rel err). Softplus is computed
as ln(1+e^x) via an ACT Exp + bit-hack log seed + one Newton step, keeping
every ACT function in the single `exp_and_others` table (no table loads).
Elementwise work is split across DVE and GPSIMD.

reference:
  t = linspace(0,1,B)[:,None]
  xt' = x0 + noise*sqrt(1-t)
  h   = relu([xt', t] @ W1.T + b1)
  ft  = tanh(h @ W2.T + b2)
  xt  = xt' + (1-t)*ft
  mu  = xt @ Wm.T + bm
  sig = softplus(xt @ Wv.T + bv)
  out = (1-t_rand)*x + t_rand*(mu + sig*eps)
"""
import numpy as np

import concourse.bacc as bacc
import concourse.tile as tile
from concourse import mybir
from concourse.bass_utils import run_bass_kernel_spmd

F32 = mybir.dt.float32
F32R = mybir.dt.float32r
I32 = mybir.dt.int32
AF = mybir.ActivationFunctionType
OP = mybir.AluOpType

B, D = 262144, 256
NCORES = 8
RS = B // NCORES          # rows per core shard = 32768
R = 512                   # rows per block
NJ = R // 128             # natural sub-tiles per block = 4
NBLK = RS // R            # 64 blocks
OC = D // 128             # feature chunks = 2

# softplus bit-hack constants: ln(z) seed = C*(bits(z) - B2), folded so that
# y0m = float(bits - BP) and ln(z) ~ C*y0m + 1
BP = 1077091419
C = float(np.log(2.0) / 2**23)

_nc_cache = {}


def build_nc():
    if "nc" in _nc_cache:
        return _nc_cache["nc"]
    nc = bacc.Bacc("TRN2")

    X0N = nc.dram_tensor("x0n", [RS, D], F32, kind="ExternalInput")
    NSN = nc.dram_tensor("nsn", [RS, D], F32, kind="ExternalInput")
    EPN = nc.dram_tensor("epn", [RS, D], F32, kind="ExternalInput")
    XNN = nc.dram_tensor("xnn", [RS, D], F32, kind="ExternalInput")
    # natural per-partition vectors: [block, vec(s1mt,trand,1-trand), j, p]
    VN = nc.dram_tensor("vn", [NBLK, 3, NJ, 128], F32, kind="ExternalInput")
    # transposed-space row vectors: [1, vec(t, 1-t), RS]
    VT = nc.dram_tensor("vt", [1, 2, RS], F32, kind="ExternalInput")
    # weights pre-transposed: [kc, p, m] with W?T[kc, p, m] = W[m, kc*128+p]
    W1AT = nc.dram_tensor("w1at", [OC, 128, D], F32, kind="ExternalInput")
    W2T = nc.dram_tensor("w2t", [OC, 128, D], F32, kind="ExternalInput")
    WMT = nc.dram_tensor("wmt", [OC, 128, D], F32, kind="ExternalInput")
    WVT = nc.dram_tensor("wvt", [OC, 128, D], F32, kind="ExternalInput")
    W1L = nc.dram_tensor("w1l", [1, D], F32, kind="ExternalInput")
    # biases in transposed space: [p, oc] with bT[p, oc] = b[oc*128+p]
    B1T = nc.dram_tensor("b1t", [128, OC], F32, kind="ExternalInput")
    B2T = nc.dram_tensor("b2t", [128, OC], F32, kind="ExternalInput")
    BMT = nc.dram_tensor("bmt", [128, OC], F32, kind="ExternalInput")
    BVT = nc.dram_tensor("bvt", [128, OC], F32, kind="ExternalInput")
    IDT = nc.dram_tensor("idt", [128, 128], F32, kind="ExternalInput")
    ONES = nc.dram_tensor("ones1", [1, 128], F32, kind="ExternalInput")
    OUTN = nc.dram_tensor("outn", [RS, D], F32, kind="ExternalOutput")

    x0_v = X0N.rearrange("(n j p) d -> n p j d", p=128, j=NJ)
    ns_v = NSN.rearrange("(n j p) d -> n p j d", p=128, j=NJ)
    ep_v = EPN.rearrange("(n j p) d -> n p j d", p=128, j=NJ)
    x_v = XNN.rearrange("(n j p) d -> n p j d", p=128, j=NJ)
    out_v = OUTN.rearrange("(n j p) d -> n p j d", p=128, j=NJ)

    with tile.TileContext(nc) as tc:
        with tc.tile_pool(name="consts", bufs=1) as cp, \
             tc.tile_pool(name="loads", bufs=2) as lp, \
             tc.tile_pool(name="work", bufs=2) as wp, \
             tc.tile_pool(name="psT", bufs=2, space="PSUM") as ppt, \
             tc.tile_pool(name="psN", bufs=2, space="PSUM") as ppn:

            # ---- one-time constants ----
            w1at = cp.tile([128, OC, D], F32R)
            w2t = cp.tile([128, OC, D], F32R)
            wmt = cp.tile([128, OC, D], F32R)
            wvt = cp.tile([128, OC, D], F32R)
            nc.sync.dma_start(out=w1at, in_=W1AT.rearrange("k p m -> p k m").bitcast(F32R))
            nc.sync.dma_start(out=w2t, in_=W2T.rearrange("k p m -> p k m").bitcast(F32R))
            nc.sync.dma_start(out=wmt, in_=WMT.rearrange("k p m -> p k m").bitcast(F32R))
            nc.sync.dma_start(out=wvt, in_=WVT.rearrange("k p m -> p k m").bitcast(F32R))
            w1l = cp.tile([1, D], F32R)
            nc.sync.dma_start(out=w1l, in_=W1L[:, :].bitcast(F32R))
            idt = cp.tile([128, 128], F32R)
            nc.sync.dma_start(out=idt, in_=IDT[:, :].bitcast(F32R))
            b1t = cp.tile([128, OC], F32)
            b2t = cp.tile([128, OC], F32)
            bmt = cp.tile([128, OC], F32)
            bvt = cp.tile([128, OC], F32)
            nc.sync.dma_start(out=b1t, in_=B1T[:, :])
            nc.sync.dma_start(out=b2t, in_=B2T[:, :])
            nc.sync.dma_start(out=bmt, in_=BMT[:, :])
            nc.sync.dma_start(out=bvt, in_=BVT[:, :])
            ones = cp.tile([1, 128], F32R)
            nc.sync.dma_start(out=ones, in_=ONES[:, :].bitcast(F32R))
            neg1 = cp.tile([128, 1], F32)
            nc.vector.memset(neg1, -1.0)

            for b in range(NBLK):
                cols = slice(b * R, (b + 1) * R)

                x0n = lp.tile([128, NJ, D], F32, tag="x0n")
                nsn = lp.tile([128, NJ, D], F32, tag="nsn")
                epn = lp.tile([128, NJ, D], F32, tag="epn")
                xnn = lp.tile([128, NJ, D], F32, tag="xnn")
                vn = lp.tile([128, 3, NJ], F32, tag="vn")
                vt = lp.tile([1, 2, R], F32R, tag="vt")
                nc.sync.dma_start(out=x0n, in_=x0_v[b])
                nc.sync.dma_start(out=nsn, in_=ns_v[b])
                nc.sync.dma_start(out=epn, in_=ep_v[b])
                nc.sync.dma_start(out=xnn, in_=x_v[b])
                nc.sync.dma_start(out=vn, in_=VN[b].rearrange("v j p -> p v j"))
                nc.sync.dma_start(out=vt, in_=VT[:, :, cols].bitcast(F32R))

                # prologue (natural): xt' = x0 + noise*sqrt(1-t)    [DVE]
                xtp = wp.tile([128, NJ, D], F32R, tag="xtp")
                for j in range(NJ):
                    nc.vector.scalar_tensor_tensor(
                        xtp[:, j, :], nsn[:, j, :], vn[:, 0, j:j + 1],
                        x0n[:, j, :], OP.mult, OP.add)

                # transpose xt' -> [feat, rows]
                pT = ppt.tile([128, OC, R], F32R, tag="tp")
                for j in range(NJ):
                    for kc in range(OC):
                        nc.tensor.matmul(
                            pT[:, kc, 128 * j:128 * (j + 1)],
                            xtp[:, j, 128 * kc:128 * (kc + 1)], idt,
                            is_transpose=True, start=True, stop=True,
                            skip_group_check=True)
                xtpT = wp.tile([128, OC, R], F32R, tag="xtpT")
                nc.scalar.activation(out=xtpT, in_=pT, func=AF.Copy)

                # layer 1: h = relu(W1a @ xt'T + w1last x t + b1)
                ph = ppt.tile([128, OC, R], F32, tag="tp")
                ht = wp.tile([128, OC, R], F32R, tag="ht")
                for oc in range(OC):
                    ocs = slice(oc * 128, (oc + 1) * 128)
                    nc.tensor.matmul(ph[:, oc, :], w1at[:, 0, ocs], xtpT[:, 0, :], start=True, stop=False)
                    nc.tensor.matmul(ph[:, oc, :], w1at[:, 1, ocs], xtpT[:, 1, :], start=False, stop=False)
                    nc.tensor.matmul(ph[:, oc, :], w1l[:, ocs], vt[:, 0, :], start=False, stop=True)
                    nc.scalar.activation(out=ht[:, oc, :], in_=ph[:, oc, :],
                                         func=AF.Relu, bias=b1t[:, oc:oc + 1])

                # layer 2: ft = tanh(W2 @ h + b2)
                pf = ppt.tile([128, OC, R], F32, tag="tp")
                ftt = wp.tile([128, OC, R], F32, tag="ftt")
                for oc in range(OC):
                    ocs = slice(oc * 128, (oc + 1) * 128)
                    nc.tensor.matmul(pf[:, oc, :], w2t[:, 0, ocs], ht[:, 0, :], start=True, stop=False)
                    nc.tensor.matmul(pf[:, oc, :], w2t[:, 1, ocs], ht[:, 1, :], start=False, stop=True)
                    nc.scalar.activation(out=ftt[:, oc, :], in_=pf[:, oc, :],
                                         func=AF.Tanh, bias=b2t[:, oc:oc + 1])

                # xtT = xt'T + (1-t)*ftT : bcast (1-t) via ones-matmul, then
                # g = ft*omb [DVE], xttT = g + xt'T [GPSIMD, in place]
                omb = ppt.tile([128, R], F32, tag="tp")
                nc.tensor.matmul(omb, ones, vt[:, 1, :], start=True, stop=True)
                g = wp.tile([128, OC, R], F32R, tag="g")
                for oc in range(OC):
                    nc.vector.tensor_tensor(g[:, oc, :], ftt[:, oc, :], omb, OP.mult)
                nc.gpsimd.tensor_tensor(g, g, xtpT, OP.add)  # g := xtT

                # heads
                pv = ppt.tile([128, OC, R], F32, tag="tp")
                pm = ppt.tile([128, OC, R], F32, tag="tp")
                xvT = wp.tile([128, OC, R], F32R, tag="xvT")
                muT = wp.tile([128, OC, R], F32R, tag="muT")
                for oc in range(OC):
                    ocs = slice(oc * 128, (oc + 1) * 128)
                    nc.tensor.matmul(pv[:, oc, :], wvt[:, 0, ocs], g[:, 0, :], start=True, stop=False)
                    nc.tensor.matmul(pv[:, oc, :], wvt[:, 1, ocs], g[:, 1, :], start=False, stop=True)
                    nc.scalar.activation(out=xvT[:, oc, :], in_=pv[:, oc, :],
                                         func=AF.Identity, bias=bvt[:, oc:oc + 1])
                    nc.tensor.matmul(pm[:, oc, :], wmt[:, 0, ocs], g[:, 0, :], start=True, stop=False)
                    nc.tensor.matmul(pm[:, oc, :], wmt[:, 1, ocs], g[:, 1, :], start=False, stop=True)
                    nc.scalar.activation(out=muT[:, oc, :], in_=pm[:, oc, :],
                                         func=AF.Identity, bias=bmt[:, oc:oc + 1])

                # transpose xv, mu back to natural
                pxv = ppn.tile([128, NJ, D], F32R, tag="np")
                for j in range(NJ):
                    for kc in range(OC):
                        nc.tensor.matmul(
                            pxv[:, j, 128 * kc:128 * (kc + 1)],
                            xvT[:, kc, 128 * j:128 * (j + 1)], idt,
                            is_transpose=True, start=True, stop=True,
                            skip_group_check=True)
                # softplus(xv) = ln(1 + e^xv), Newton refined
                w = wp.tile([128, NJ, D], F32, tag="w")
                nc.scalar.activation(out=w, in_=pxv, func=AF.Exp)
                z = wp.tile([128, NJ, D], F32, tag="z")
                nc.vector.tensor_scalar_add(z, w, 1.0)
                y0m = wp.tile([128, NJ, D], F32, tag="y0m")
                nc.vector.tensor_scalar(y0m, z.bitcast(I32), BP, None, OP.subtract)
                e = wp.tile([128, NJ, D], F32, tag="e")
                nc.scalar.activation(out=e, in_=y0m, func=AF.Exp, bias=neg1[:, :], scale=-C)
                nc.gpsimd.tensor_tensor(z, z, e, OP.mult)  # z := u = z*exp(-y0)
                sg = wp.tile([128, NJ, D], F32, tag="sg")
                nc.vector.scalar_tensor_tensor(sg, y0m, C, z, OP.mult, OP.add)
                # sg := sigma * eps
                nc.gpsimd.tensor_tensor(sg, sg, epn, OP.mult)

                pmu = ppn.tile([128, NJ, D], F32R, tag="np")
                for j in range(NJ):
                    for kc in range(OC):
                        nc.tensor.matmul(
                            pmu[:, j, 128 * kc:128 * (kc + 1)],
                            muT[:, kc, 128 * j:128 * (j + 1)], idt,
                            is_transpose=True, start=True, stop=True,
                            skip_group_check=True)
                # m2 = mu + sigma*eps   (in place on sg)
                nc.vector.tensor_tensor(sg, sg, pmu, OP.add)

                # blend: out = x*(1-tr) + tr*m2
                xo = wp.tile([128, NJ, D], F32, tag="xo")
                outt = wp.tile([128, NJ, D], F32, tag="outt")
                for j in range(NJ):
                    nc.vector.tensor_scalar_mul(xo[:, j, :], xnn[:, j, :], vn[:, 2, j:j + 1])
                    nc.vector.scalar_tensor_tensor(
                        outt[:, j, :], sg[:, j, :], vn[:, 1, j:j + 1],
                        xo[:, j, :], OP.mult, OP.add)

                nc.sync.dma_start(out=out_v[b], in_=outt)

    nc.finalize()
    _nc_cache["nc"] = nc
    return nc


def _prep_inputs(x, noise, x0, t_rand, eps, W1, b1, W2, b2, Wm, bm, Wv, bv):
    """Shard on host; returns in_maps for the 8 cores."""
    t = np.linspace(0.0, 1.0, B, dtype=np.float32)
    s1mt = np.sqrt(1.0 - t, dtype=np.float32)
    omt = (1.0 - t).astype(np.float32)
    tr = np.ascontiguousarray(t_rand[:, 0])
    otr = (1.0 - tr).astype(np.float32)

    def wT(W):  # [D, D] -> [OC, 128, D] with out[k, p, m] = W[m, k*128+p]
        return np.ascontiguousarray(W.T.reshape(OC, 128, D))

    def bT(b):  # [D] -> [128, OC]
        return np.ascontiguousarray(b.reshape(OC, 128).T)

    shared = {
        "w1at": wT(np.ascontiguousarray(W1[:, :D])),
        "w2t": wT(W2), "wmt": wT(Wm), "wvt": wT(Wv),
        "w1l": np.ascontiguousarray(W1[:, D]).reshape(1, D),
        "b1t": bT(b1), "b2t": bT(b2), "bmt": bT(bm), "bvt": bT(bv),
        "idt": np.eye(128, dtype=np.float32),
        "ones1": np.ones((1, 128), dtype=np.float32),
    }

    in_maps = []
    for c in range(NCORES):
        sl = slice(c * RS, (c + 1) * RS)
        vn = np.stack([v[sl].reshape(NBLK, NJ, 128) for v in (s1mt, tr, otr)], axis=1)
        vt = np.stack([t[sl], omt[sl]], axis=0)[None]
        in_maps.append({
            "x0n": x0[sl], "nsn": noise[sl], "epn": eps[sl], "xnn": x[sl],
            "vn": np.ascontiguousarray(vn), "vt": np.ascontiguousarray(vt),
            **shared,
        })
    return in_maps


def _run(in_maps, trace=False):
    nc = build_nc()
    return run_bass_kernel_spmd(nc, in_maps, list(range(NCORES)), trace=trace)


def _assemble(results):
    out = np.empty((B, D), dtype=np.float32)
    for c in range(NCORES):
        out[c * RS:(c + 1) * RS, :] = results[c]["outn"]
    return out


def kernel(**inputs) -> np.ndarray:
    in_maps = _prep_inputs(**inputs)
    res = _run(in_maps, trace=False)
    return _assemble(res.results)


def kernel_traced(**inputs):
    """Same as kernel() but with NTFF tracing; returns (out, exec_time_ns, results)."""
    in_maps = _prep_inputs(**inputs)
    res = _run(in_maps, trace=True)
    return _assemble(res.results), res.exec_time_ns, res
